# revision 1
# baseline (speedup 1.0000x reference)
"""Bilateral filter (3x3, sigma=0.8) Trainium2 Bass kernel. v3

Sharding: fully data-parallel over the fused batch B*V = 8 -> one
(C=3,H=512,W=512) image per NeuronCore, 8 cores.

Per-core layout: H=512 rows split 4 rows/partition over 128 partitions.
Each partition holds 5 rows (4 data rows + 1 halo row below) x 520 cols
(2 left pad, 512 data, 6 right pad) flattened in the free dimension, so
every 3x3 tap at +e is a constant flat offset. Fields are computed on a
4-row grid (one value per output pixel); the -e (upward) taps that cross
a partition boundary are accumulated on the TensorEngine with partition-
shift matrices sh* = eye(k=1)*ws (vs recomputing a halo field row as the
5/6-row variants do), cutting vector+scalar field work by ~20%.

Math (unnormalized weights; the per-pixel wd/wc normalizations cancel in
num/den; the 1e-7 eps is dropped, |effect| ~1e-4):
  For e in {(0,1),(1,0),(1,1),(1,-1)} (pair symmetry covers -e):
    g_e  = DErf(sqrt(S)(d(+e)-d))            # (2/sqrt(pi)) exp(-S dd^2)
    G_e  = sum_c DErf(sqrt(S)(c_c(+e)-c_c))
    F_e  = g_e * G_e                         # ws_e folded into PE weights
    den += ws_e*(F_e*m(+e) @0 + F_e*m @-e)   (+ center 3*ws0*(4/pi)*m)
    num_c likewise via chained products (F*m(+e))*c(+e) and (F*m)*c.
  The DErf constant (2/sqrt(pi))^2 is uniform across taps once the
  center is scaled by 4/pi, so it cancels in num/den.
Tap accumulation runs on the TensorEngine (scaled-identity matmuls into
PSUM); elementwise work runs on Vector with a few slack-tolerant leaf
products optionally on GpSimd.
"""

import math
import numpy as np
import sys

if "/opt/trn_rl_repo" not in sys.path:
    sys.path.insert(0, "/opt/trn_rl_repo")

import concourse.bass as bass
import concourse.tile as tile
from concourse import mybir
from concourse.bass_utils import run_bass_kernel_spmd

# ---- problem constants (hardcoded per spec) ----
B, V, C, H, W = 2, 4, 3, 512, 512
N_CORES = 8
KS = 3
SIG = 0.3 * ((KS - 1) * 0.5 - 1) + 0.8           # 0.8
S = 1.0 / (2.0 * SIG * SIG)                       # 0.78125

# spatial gaussian, normalized
_xs = np.arange(KS, dtype=np.float64)
_gx, _gy = np.meshgrid(_xs, _xs, indexing="xy")
_w = np.exp(-(((_gx - 1) ** 2 + (_gy - 1) ** 2)) * S)
_w = _w / _w.sum()
W0 = float(_w[1, 1])   # center
W1 = float(_w[0, 1])   # edge-adjacent
W2 = float(_w[0, 0])   # diagonal

# layout constants
R = 4                  # data rows per partition
W2C = 520              # row stride (2 left pad + 512 data + 6 right pad)
NROW = 5               # rows per partition incl. bottom halo
FLAT = NROW * W2C      # 2600
ALLOC = FLAT + 24      # slack so reads at +521 from flat 2079 stay in-bounds
PROD = R * W2C         # 2080: field/product grid (4 rows, all cols)
COL0 = 2               # first data col

# (er, ec, flat offset, spatial weight index)
ES = [(0, 1, 1, 0), (1, 0, W2C, 0), (1, 1, W2C + 1, 1), (1, -1, W2C - 1, 1)]
SQS = math.sqrt(S)          # DErf(SQS*x) = 2/sqrt(pi) * exp(-S x^2)
PHI2 = 4.0 / math.pi        # (2/sqrt(pi))^2, folded into the center tap
WCEN = 3.0 * W0 * PHI2

# engine assignment switches
USE_GP_Z = False            # Z products for dirs 2,3 on GpSimd
USE_GP_NCC = False          # ncc muls on GpSimd
USE_CCE_G = False           # G-sum adds on DMA CCE instead of vector
N_WARM = 32                 # PE warm-up dummy matmuls (0 = off)

F16 = mybir.dt.float16
F32 = mybir.dt.float32
AF = mybir.ActivationFunctionType
ALU = mybir.AluOpType

# weight-matrix slots in the idents tile
ID_PLAIN, ID_W1, ID_W2, ID_CEN, ID_SH1, ID_SH2, ID_NEG = range(7)


# ---- walrus single-wait workaround ----------------------------------------
# This container's walrus accepts only ONE sync_info.on_wait per instruction;
# Tile emits multi-wait instructions. Hoist all but the last wait onto
# injected single-wait NoOps just before the original.
import orjson as _orjson

_SCRATCH = "wsplit_scratch"


def _mk_nop(name, engine, wait):
    return {"name": name, "engine": engine, "ins": [], "outs": [],
            "opcode": "NoOp",
            "sync_info": {"on_wait": [wait], "on_update": []}}


def _ldw_sig(ins):
    aps = ins.get("ins") or []
    if not aps:
        return None
    a = aps[0]
    return (a.get("memref"), a.get("offset"), str(a.get("ap")), a.get("dtype"))


def _dedup_ldweights(m):
    """NoOp-ify PE Ldweights whose weights are already loaded (same static
    source AP as the previous Ldweights, sourced from the idents tile).
    Sync info is preserved on the NoOp."""
    for f in m.get("functions", []):
        for bb in f.get("blocks", []):
            last = None
            for ins in bb.get("instructions", []):
                if ins.get("opcode") != "Ldweights":
                    continue
                sig = _ldw_sig(ins)
                if (sig is not None and sig == last
                        and sig[0] and "idents" in sig[0]):
                    ins["opcode"] = "NoOp"
                    ins["ins"] = []
                    ins["outs"] = []
                else:
                    last = sig
    return m


def _split_multiwaits(bir_bytes):
    m = _orjson.loads(bir_bytes)
    _dedup_ldweights(m)
    for f in m.get("functions", []):
        for bb in f.get("blocks", []):
            out = []
            for ins in bb.get("instructions", []):
                si = ins.get("sync_info")
                waits = (si or {}).get("on_wait") or []
                if len(waits) > 1:
                    for k, w in enumerate(waits[:-1]):
                        nm = f"{ins['name']}-wsplit{k}"
                        out.append(_mk_nop(nm, ins["engine"], w))
                    si["on_wait"] = [waits[-1]]
                out.append(ins)
            bb["instructions"] = out
    return _orjson.dumps(m)


_BUILD_CACHE = {}


def _build_nc():
    nc = bass.Bass()
    x_in = nc.declare_dram_parameter("x", [5, 128, NROW, W], F16, isOutput=False)
    id_in = nc.declare_dram_parameter("ident", [7, 128, 128], F16, isOutput=False)
    o_out = nc.declare_dram_parameter("out", [C, H, W], F16, isOutput=True)
    nc.dram_tensor(_SCRATCH, [4], F32)

    with tile.TileContext(nc) as tc:
        _emit(nc, tc, x_in, id_in, o_out)

    orig_to_json = nc.to_json_bytes
    nc.to_json_bytes = lambda: _split_multiwaits(orig_to_json())
    return nc


def _emit(nc, tc, x_in, id_in, o_out):
    from contextlib import ExitStack
    ctx = ExitStack()
    with ctx:
        persist = ctx.enter_context(tc.tile_pool(name="persist", bufs=1))
        tdp = ctx.enter_context(tc.tile_pool(name="tdp", bufs=2))
        tcap = ctx.enter_context(tc.tile_pool(name="tcap", bufs=2))
        fp = ctx.enter_context(tc.tile_pool(name="fp", bufs=2))
        yz_p = ctx.enter_context(tc.tile_pool(name="yz", bufs=4))
        fin_p = ctx.enter_context(tc.tile_pool(name="fin", bufs=2))
        psum_p = ctx.enter_context(
            tc.tile_pool(name="psum", bufs=1, space=bass.MemorySpace.PSUM)
        )

        # ---- persistent fp16 planes / fields ----
        d16 = persist.tile([128, ALLOC], F16, tag="d16", name="d16")
        m16 = persist.tile([128, ALLOC], F16, tag="m16", name="m16")
        c16all = persist.tile([128, C, ALLOC], F16, tag="c16all", name="c16all")
        c16 = [c16all[:, i, :] for i in range(C)]
        idents = persist.tile([128, 7, 128], F16, tag="idents", name="idents")
        wmat = [idents[:, j, :] for j in range(7)]
        ftm = [persist.tile([128, PROD], F16, tag=f"ftm{i}", name=f"ftm{i}") for i in range(4)]
        fhm = [persist.tile([128, PROD], F16, tag=f"fhm{i}", name=f"fhm{i}") for i in range(4)]
        m3w0 = persist.tile([128, PROD], F16, tag="m3w0", name="m3w0")
        ncc = persist.tile([128, C, PROD], F16, tag="ncc", name="ncc")
        r16 = persist.tile([128, R, W], F16, tag="r16", name="r16")
        lden = persist.tile([128, R, W], F32, tag="lden", name="lden")

        def v3(ap_flat):  # [p, row, col] view of a flat plane
            return ap_flat[:, 0:FLAT].rearrange("p (a b) -> p a b", b=W2C)

        # zero pad columns and the slack tail once
        for t in (d16, m16, *c16):
            nc.vector.memset(v3(t[:])[:, :, 0:COL0], 0.0)
            nc.vector.memset(v3(t[:])[:, :, COL0 + W:W2C], 0.0)
            nc.vector.memset(t[:, FLAT:ALLOC], 0.0)

        # ---- load the 5 fp16 planes (rows 4p..4p+4, built host-side) ----
        planes = [d16, c16[0], c16[1], c16[2], m16]
        engs = [nc.sync, nc.scalar, nc.sync, nc.scalar, nc.sync]
        for k, dst16 in enumerate(planes):
            engs[k].dma_start(
                v3(dst16[:])[:, 0:NROW, COL0:COL0 + W], x_in[k]
            )
        nc.sync.dma_start(idents[:], id_in.rearrange("j p c -> p j c"))

        den = psum_p.tile([128, R, W], F32, tag="acc", name="den", bufs=2)

        # PE warm-up: matmuls into den bank 0 while DMAs run; the HAM clock
        # gate needs ~3.4us of sustained PE activity to lift the PE from 1.2
        # to 2.4 GHz. The real accumulation's start=True overwrites bank 0.
        scratch = persist.tile([128, W], F16, tag="pewarm", name="pewarm")
        nc.vector.memset(scratch[:], 0.0)
        for k in range(N_WARM):
            nc.tensor.matmul(
                den[:, 0, :], wmat[ID_PLAIN], scratch[:],
                start=True, stop=True, skip_group_check=True,
            )

        nc.vector.tensor_scalar_mul(m3w0[:], m16[:, 0:PROD], WCEN)

        def mm(acc, wi, rhs_flat, off, row, start=False, stop=False,
               n=W, ocol=0):
            nc.tensor.matmul(
                acc[:, row, ocol:ocol + n], wmat[wi],
                rhs_flat[:, off:off + n],
                start=start, stop=stop,
            )

        def accum_dir(acc, i, a_t, b_t, first, deferred):
            """acc += ws_e*(A-term at 0) + ws_e*(B-term at -e) for dir i.
            B windows skip the boundary column (where the halo tap is zero);
            sh* matmuls are deferred so weight loads dedup."""
            er, ec, ef, iw = ES[i]
            widn = ID_W1 if iw == 0 else ID_W2
            wsh = ID_SH1 if iw == 0 else ID_SH2
            for r in range(R):
                mm(acc, widn, a_t, r * W2C + COL0, r, start=first)
            n = W - abs(ec)
            ocol = max(0, ec)
            icol = COL0 + max(0, -ec)
            if er == 0:
                for r in range(R):
                    mm(acc, widn, b_t, r * W2C + icol, r, n=n, ocol=ocol)
            else:
                for r in range(1, R):
                    mm(acc, widn, b_t, (r - 1) * W2C + icol, r, n=n, ocol=ocol)
                deferred.append((wsh, b_t, 3 * W2C + icol, n, ocol))

        def flush_sh(acc, deferred):
            deferred.sort(key=lambda t: t[0])
            for wsh, b_t, off, n, ocol in deferred:
                mm(acc, wsh, b_t, off, 0, n=n, ocol=ocol)
            deferred.clear()

        # ---- phase A: per-dir fields + den accumulation; ch0 products ----
        yz0 = []
        den_sh = []
        for i, (er, ec, ef, iw) in enumerate(ES):
            td = tdp.tile([128, PROD], F16, tag="td", name="td")
            nc.vector.tensor_sub(td[:], d16[:, ef:PROD + ef], d16[:, 0:PROD])
            nc.scalar.activation(td[:], td[:], AF.Derivative_Erf, scale=SQS)

            tca = tcap.tile([128, C, PROD], F16, tag="tca", name="tca")
            nc.vector.tensor_sub(
                tca[:], c16all[:, :, ef:PROD + ef], c16all[:, :, 0:PROD]
            )
            G = fp.tile([128, PROD], F16, tag="G", name="G")
            if USE_CCE_G:
                # exp of ch0 lands in G; chs 1-2 accumulate via DMA CCE adds
                nc.scalar.activation(G[:], tca[:, 0, :],
                                     AF.Derivative_Erf, scale=SQS)
                nc.scalar.activation(tca[:, 1:3, :], tca[:, 1:3, :],
                                     AF.Derivative_Erf, scale=SQS)
                nc.gpsimd.dma_start(G[:], tca[:, 1, :], accum_op=ALU.add)
                nc.gpsimd.dma_start(G[:], tca[:, 2, :], accum_op=ALU.add)
            else:
                nc.scalar.activation(tca[:], tca[:],
                                     AF.Derivative_Erf, scale=SQS)
                nc.vector.tensor_add(G[:], tca[:, 0, :], tca[:, 1, :])
                nc.vector.tensor_add(G[:], G[:], tca[:, 2, :])
            F = fp.tile([128, PROD], F16, tag="F", name="F")
            nc.vector.tensor_mul(F[:], td[:], G[:])
            nc.vector.tensor_mul(ftm[i][:], F[:], m16[:, ef:PROD + ef])
            nc.vector.tensor_mul(fhm[i][:], F[:], m16[:, 0:PROD])

            accum_dir(den, i, ftm[i][:], fhm[i][:], i == 0, den_sh)

            # ncc_c = m3w0 * c_c as a leaf op
            if i < C:
                nc.vector.tensor_mul(ncc[:, i, :], m3w0[:], c16[i][:, 0:PROD])

            # channel-0 products early (accumulated in phase B)
            Y = yz_p.tile([128, PROD], F16, tag="Y", name="Y")
            Z = yz_p.tile([128, PROD], F16, tag="Z", name="Z")
            nc.vector.tensor_mul(Y[:], ftm[i][:], c16[0][:, ef:PROD + ef])
            nc.vector.tensor_mul(Z[:], fhm[i][:], c16[0][:, 0:PROD])
            yz0.append((Y, Z))

        flush_sh(den, den_sh)
        # den center: + WCEN * m  (scaled identity)
        for r in range(R):
            mm(den, ID_CEN, m16[:], r * W2C + COL0, r, stop=(r == R - 1))

        # 1/den = exp(-ln(den)); den>0 (products of exps, positive mask).
        nc.scalar.activation(lden[:], den[:], AF.Ln)
        nc.scalar.activation(r16[:], lden[:], AF.Exp, scale=-1.0)

        def _finals(num, ci):
            n16 = fin_p.tile([128, R, W], F16, tag="n16", name="n16")
            nc.scalar.activation(n16[:], num[:], AF.Copy)
            o16 = fin_p.tile([128, R, W], F16, tag="o16", name="o16")
            nc.vector.tensor_mul(o16[:], n16[:], r16[:])
            nc.sync.dma_start(
                o_out[ci].rearrange("(p r) w -> p r w", r=R), o16[:]
            )

        # ---- phase B: per-channel numerators ----
        pending = None
        for ci in range(C):
            num = psum_p.tile([128, R, W], F32, tag="acc",
                              name=f"num{ci}", bufs=2)
            num_sh = []
            for i, (er, ec, ef, iw) in enumerate(ES):
                if ci == 0:
                    Y, Z = yz0[i]
                else:
                    Y = yz_p.tile([128, PROD], F16, tag="Y", name="Y")
                    Z = yz_p.tile([128, PROD], F16, tag="Z", name="Z")
                    nc.vector.tensor_mul(
                        Y[:], ftm[i][:], c16[ci][:, ef:PROD + ef]
                    )
                    nc.vector.tensor_mul(Z[:], fhm[i][:], c16[ci][:, 0:PROD])
                accum_dir(num, i, Y[:], Z[:], i == 0, num_sh)
            flush_sh(num, num_sh)
            for r in range(R):
                mm(num, ID_PLAIN, ncc[:, ci, :], r * W2C + COL0, r,
                   stop=(r == R - 1))
            if pending is not None:
                _finals(*pending)
            pending = (num, ci)
        _finals(*pending)


def _get_nc():
    if "nc" not in _BUILD_CACHE:
        _BUILD_CACHE["nc"] = _build_nc()
    return _BUILD_CACHE["nc"]


def _halo_planes(d, c, m):
    """[N,5,128,5,512] fp16: per-partition rows 4p..4p+4, zero bottom halo."""
    from numpy.lib.stride_tricks import as_strided
    stack = np.empty((N_CORES, 5, H + 4, W), np.float16)
    for i in range(N_CORES):
        for k, arr in enumerate((d[i], c[i, 0], c[i, 1], c[i, 2], m[i])):
            stack[i, k, 0:H] = arr
    stack[:, :, H:H + 4] = 0.0
    s = stack.strides
    win = as_strided(
        stack,
        shape=(N_CORES, 5, 128, NROW, W),
        strides=(s[0], s[1], 4 * s[2], s[2], s[3]),
    )
    return np.ascontiguousarray(win)


def _identities():
    eye = np.eye(128)
    sh = np.eye(128, k=1)
    return np.stack([
        eye, eye * W1, eye * W2, eye * WCEN, sh * W1, sh * W2, -eye,
    ]).astype(np.float16)


def _run(depth, color, mask, trace=False, **kw):
    nc = _get_nc()
    d = np.asarray(depth, dtype=np.float32).reshape(N_CORES, H, W)
    c = np.asarray(color, dtype=np.float32).reshape(N_CORES, C, H, W)
    m = np.asarray(mask, dtype=np.float32).reshape(N_CORES, H, W)
    x16 = _halo_planes(d, c, m)
    ids = _identities()
    in_maps = [{"x": x16[i], "ident": ids} for i in range(N_CORES)]
    res = run_bass_kernel_spmd(
        nc, in_maps, list(range(N_CORES)), trace=trace, **kw
    )
    out = np.stack([np.asarray(res.results[i]["out"]) for i in range(N_CORES)])
    return out.reshape(B, V, C, H, W).astype(np.float32), res


def kernel(depth, color, mask):
    out, _ = _run(depth, color, mask, trace=False)
    return out

